# revision 4
# baseline (speedup 1.0000x reference)
"""Trainium2 Bass kernel for the DiscretizedDPLRSSMBlock problem.

Computes, for h, x of shape [4096, 4096] (batch, hidden):

    out = h + (h * a_diag + (h @ q_vec) @ p_vec.T) + x @ b_mat        (DELTA = 1.0)
        = h * (1 + a_diag) + (h @ q_vec) @ p_vec.T + x @ b_mat

Sharding: data-parallel over the batch axis across 8 NeuronCores (512 rows
per core); a_diag/p_vec/q_vec/b_mat replicated.

Per-core kernel works in a transposed layout (hidden on partitions):
    outT[n, m] = sum_k B[k, n] * xT[k, m]        (x @ B, B tiles are the
                                                  stationary matmul operand
                                                  in natural DRAM layout)
               + sum_r p[n, r] * hqT[r, m]       (rank-4 term, hqT = q^T hT)
               + (1 + a[n]) * hT[n, m]           (per-partition scalar on DVE)

All matmul operands are bf16 (fp32 PSUM accumulation); output is fp32.
"""

import numpy as np
import ml_dtypes

import concourse.mybir as mybir
import concourse.tile as tile
from concourse import bacc
from concourse.bass_utils import run_bass_kernel_spmd

HIDDEN = 4096
BATCH = 4096
RANK = 4
N_CORES = 8
MB = BATCH // N_CORES  # 512 batch rows per core
P = 128
KT = HIDDEN // P       # 32 contraction tiles
NT = HIDDEN // P       # 32 output row tiles (hidden)
NCHUNK = 4             # resident tensors split into 4 DMA chunks
CH = KT // NCHUNK      # 8 k-tiles per chunk
NGROUP = NT // 8       # 4 n-tiles per b-column streaming group (512 cols)

BF16 = mybir.dt.bfloat16
F32 = mybir.dt.float32


def build_bass():
    """Build the single-core Tile program (same program runs SPMD on all 8)."""
    nc = bacc.Bacc("TRN2", target_bir_lowering=False, debug=False)

    b = nc.dram_tensor("b", [HIDDEN, HIDDEN], BF16, kind="ExternalInput")
    xT = nc.dram_tensor("xT", [HIDDEN, MB], BF16, kind="ExternalInput")
    hT = nc.dram_tensor("hT", [HIDDEN, MB], BF16, kind="ExternalInput")
    q = nc.dram_tensor("q", [HIDDEN, RANK], BF16, kind="ExternalInput")
    pT = nc.dram_tensor("pT", [RANK, HIDDEN], BF16, kind="ExternalInput")
    a_r = nc.dram_tensor("a_r", [P, NT], F32, kind="ExternalInput")
    outT = nc.dram_tensor("outT", [HIDDEN, MB], F32, kind="ExternalOutput")

    b_r = b.rearrange("(t p) n -> p t n", p=P)     # [128, 32, 4096]
    xT_r = xT.rearrange("(t p) m -> p t m", p=P)   # [128, 32, 512]
    hT_r = hT.rearrange("(t p) m -> p t m", p=P)
    q_r = q.rearrange("(t p) r -> p t r", p=P)     # [128, 32, 4]

    with (
        tile.TileContext(nc) as tc,
        tc.tile_pool(name="const", bufs=1) as cpool,
        tc.tile_pool(name="bcols", bufs=3) as bpool,
        tc.tile_pool(name="psum", bufs=4, space="PSUM") as pspool,
        tc.tile_pool(name="outs", bufs=4) as opool,
    ):
        # Resident inputs, chunked so early matmuls can start before the
        # whole tensor has landed.
        xc, hc = [], []
        for c in range(NCHUNK):
            t0 = c * CH
            xt = cpool.tile([P, CH, MB], BF16, tag=f"x{c}")
            nc.sync.dma_start(xt[:], xT_r[:, t0 : t0 + CH, :])
            xc.append(xt)
            ht = cpool.tile([P, CH, MB], BF16, tag=f"h{c}")
            nc.sync.dma_start(ht[:], hT_r[:, t0 : t0 + CH, :])
            hc.append(ht)

        q_sb = cpool.tile([P, KT, RANK], BF16, tag="q")
        nc.sync.dma_start(q_sb[:], q_r[:])
        pT_sb = cpool.tile([RANK, HIDDEN], BF16, tag="pT")
        nc.sync.dma_start(pT_sb[:], pT[:, :])
        araw = cpool.tile([P, NT], F32, tag="araw")
        nc.sync.dma_start(araw[:], a_r[:, :])
        a1 = cpool.tile([P, NT], F32, tag="a1")
        nc.vector.tensor_scalar_add(a1[:], araw[:], 1.0)

        # hqT = q^T @ hT : [4, 512]
        hq_ps = pspool.tile([RANK, MB], F32, tag="hq", bufs=1)
        for t in range(KT):
            nc.tensor.matmul(
                hq_ps[:],
                q_sb[:, t],
                hc[t // CH][:, t % CH],
                start=(t == 0),
                stop=(t == KT - 1),
            )
        hq_sb = cpool.tile([RANK, MB], BF16, tag="hq_sb")
        nc.vector.tensor_copy(hq_sb[:], hq_ps[:])

        # Main loop: stream B 512 columns at a time; 4 output tiles per group.
        for g in range(NT // NGROUP):
            n0 = g * NGROUP * P  # first hidden column of this group
            bcs = []
            for c in range(NCHUNK):
                bc = bpool.tile([P, CH, NGROUP * P], BF16, tag=f"b{c}")
                nc.sync.dma_start(
                    bc[:], b_r[:, c * CH : (c + 1) * CH, n0 : n0 + NGROUP * P]
                )
                bcs.append(bc)
            for sub in range(NGROUP):
                tn = g * NGROUP + sub
                ps = pspool.tile([P, MB], F32, tag="ps")
                for t in range(KT):
                    nc.tensor.matmul(
                        ps[:],
                        bcs[t // CH][:, t % CH, sub * P : (sub + 1) * P],
                        xc[t // CH][:, t % CH],
                        start=(t == 0),
                        stop=False,
                    )
                nc.tensor.matmul(
                    ps[:],
                    pT_sb[:, tn * P : (tn + 1) * P],
                    hq_sb[:],
                    start=False,
                    stop=True,
                )
                ot = opool.tile([P, MB], F32, tag="ot")
                nc.vector.scalar_tensor_tensor(
                    ot[:],
                    hc[tn // CH][:, tn % CH],
                    a1[:, tn : tn + 1],
                    ps[:],
                    mybir.AluOpType.mult,
                    mybir.AluOpType.add,
                )
                nc.sync.dma_start(outT[tn * P : (tn + 1) * P, :], ot[:])

    nc.compile()
    return nc


_NC_CACHE = []


def _get_nc():
    if not _NC_CACHE:
        _NC_CACHE.append(build_bass())
    return _NC_CACHE[0]


LAST_RESULTS = []  # stash of the last BassKernelResults, for test harnesses


def make_in_maps(h, x, a_diag, p_vec, q_vec, b_mat):
    """Shard + lay out the full inputs into per-core in_maps."""
    h = np.asarray(h, dtype=np.float32)
    x = np.asarray(x, dtype=np.float32)
    a_diag = np.asarray(a_diag, dtype=np.float32)
    p_vec = np.asarray(p_vec, dtype=np.float32)
    q_vec = np.asarray(q_vec, dtype=np.float32)
    b_mat = np.asarray(b_mat, dtype=np.float32)

    bf = ml_dtypes.bfloat16
    b_bf = np.ascontiguousarray(b_mat.astype(bf))
    q_bf = np.ascontiguousarray(q_vec.astype(bf))
    pT_bf = np.ascontiguousarray(p_vec.T.astype(bf))
    # a_r[p, t] = a_diag[t*128 + p]
    a_r = np.ascontiguousarray(a_diag.reshape(NT, P).T)

    in_maps = []
    for c in range(N_CORES):
        sl = slice(c * MB, (c + 1) * MB)
        in_maps.append(
            {
                "b": b_bf,
                "xT": np.ascontiguousarray(x[sl].T.astype(bf)),
                "hT": np.ascontiguousarray(h[sl].T.astype(bf)),
                "q": q_bf,
                "pT": pT_bf,
                "a_r": a_r,
            }
        )
    return in_maps


def _axon_device_reset():
    """Best-effort heal of a wedged axon-tunneled device (NRT_EXEC_UNIT_
    UNRECOVERABLE). No-op when the axon .so isn't present."""
    try:
        import ctypes

        lib = ctypes.CDLL("/opt/axon/libaxon_pjrt.so")
        lib.axon_reset.restype = ctypes.c_int64
        lib.axon_reset()
    except Exception:
        pass


def kernel(h, x, a_diag, p_vec, q_vec, b_mat, trace=False):
    nc = _get_nc()
    in_maps = make_in_maps(h, x, a_diag, p_vec, q_vec, b_mat)
    try:
        res = run_bass_kernel_spmd(
            nc, in_maps, core_ids=list(range(N_CORES)), trace=trace
        )
    except Exception as e:
        if "UNRECOVERABLE" not in str(e) and "UNAVAILABLE" not in str(e):
            raise
        _axon_device_reset()
        res = run_bass_kernel_spmd(
            nc, in_maps, core_ids=list(range(N_CORES)), trace=trace
        )
    LAST_RESULTS.clear()
    LAST_RESULTS.append(res)

    out = np.empty((BATCH, HIDDEN), dtype=np.float32)
    for c in range(N_CORES):
        out[c * MB : (c + 1) * MB, :] = res.results[c]["outT"].T
    return out
